# revision 26
# baseline (speedup 1.0000x reference)
"""Causal multi-head attention block (B=16, S=1024, d=1024, H=16) on 8 NeuronCores.

Strategy: data-parallel over batch (2 batches per core), no collectives.
Host pre-transposes + fp16-casts x and the four weight matrices, so the
device kernel has no transpose phase at all.

Per-core kernel (fp16 matmuls, fp32 accumulation):
  proj:  QT/KT = W-tiles @ xT (transposed layout [d_out, m]),
         V = x @ Wv.T packed in 65-wide per-head strips with a fused
         ones column (attn-out matmuls then also produce row sums).
  attn:  head-pair outer, q-chunk inner. Rounds of 2 k-tiles x 2 heads:
         scoresT[k, q] via row-tiled concurrent PE matmuls (head even on
         partitions 0-63, head odd on 64-127) into a [128, 2048] PSUM
         strip, one batched ACT exp per strip, causal diag masking via
         0/1 triangle multiplies on DVE, attn-out accumulation on PE.
         Softmax denominators: sum rows gathered to SBUF, one batched
         DVE reciprocal per batch over all 32 (head, q-chunk) rows, DMA
         partition-broadcast, in-place DVE normalize of AO.
  out:   y = AO.T @ WoT per 128-row m-tile, DVE copy, DMA to DRAM.

The attention rounds stall the PE for ~2us each on the exp result, so
independent projection / output-projection "filler" units (8 accumulate
matmuls + a copy each) are interleaved one-per-round to keep the PE busy
and the HAM clock warm. Because head pair p only needs its own Q/K
d_out-tile and its own V half, attention for heads 0-7 of batch b can
run while the B-half projections of the same batch are still in flight:
  proj A(b0) -> attn(b0) [filler: proj B(b0), proj A(b1)]
             -> attn(b1) [filler: proj B(b1), out-proj(b0)]
             -> out-proj(b1).
Biases bq/bk are zero by problem spec (ignored); bv/bo folded in
exactly on the host (y += bv @ Wo.T + bo).
"""

import numpy as np

_CACHE: dict = {}

S = 1024
D = 1024
H = 16
DH = 64
BPC = 2           # batches per core
M = BPC * S       # tokens per core
NCORES = 8
NDT = D // 128    # 8 d-tiles


def _build_nc():
    import concourse.bass as bass
    import concourse.mybir as mybir
    import concourse.tile as tile
    from concourse import bacc
    from contextlib import ExitStack
    from collections import deque

    f32 = mybir.dt.float32
    f16 = mybir.dt.float16
    EXPF = mybir.ActivationFunctionType.Exp

    nc = bacc.Bacc("TRN2", target_bir_lowering=False, debug=False,
                   num_devices=NCORES)

    xT_d = nc.dram_tensor("xT", [D, M], f16, kind="ExternalInput")
    wq_d = nc.dram_tensor("WqT", [D, D], f16, kind="ExternalInput")
    wk_d = nc.dram_tensor("WkT", [D, D], f16, kind="ExternalInput")
    wv_d = nc.dram_tensor("WvT", [D, D], f16, kind="ExternalInput")
    wo_d = nc.dram_tensor("WoT", [D, D], f16, kind="ExternalInput")
    tri_d = nc.dram_tensor("tri01", [128, 128], f16, kind="ExternalInput")
    y_d = nc.dram_tensor("y", [M, D], f32, kind="ExternalOutput")

    with tile.TileContext(nc) as tc, ExitStack() as top:
        consts = top.enter_context(tc.tile_pool(name="consts", bufs=1))
        persist = top.enter_context(tc.tile_pool(name="persist", bufs=1))
        wpool = top.enter_context(tc.tile_pool(name="wpool", bufs=2))
        expp = top.enter_context(tc.tile_pool(name="expp", bufs=3))
        sumsp = top.enter_context(tc.tile_pool(name="sumsp", bufs=2))
        stp = top.enter_context(tc.tile_pool(name="stp", bufs=2))
        st32p = top.enter_context(tc.tile_pool(name="st32p", bufs=1))
        rbp = top.enter_context(tc.tile_pool(name="rbp", bufs=2))
        tmpp = top.enter_context(tc.tile_pool(name="tmpp", bufs=2))
        ysp = top.enter_context(tc.tile_pool(name="ysp", bufs=2))
        psS = top.enter_context(tc.tile_pool(name="psS", bufs=2, space="PSUM"))
        psO = top.enter_context(tc.tile_pool(name="psO", bufs=1, space="PSUM"))
        psP = top.enter_context(tc.tile_pool(name="psP", bufs=2, space="PSUM"))

        tri01 = consts.tile([128, 128], f16, tag="tri")
        nc.sync.dma_start(out=tri01, in_=tri_d[:, :])

        # persistent activations (fp16)
        xTs = persist.tile([128, NDT, M], f16, tag="xTs")
        QT = persist.tile([128, NDT, M], f16, tag="QT")    # [d_out, m]
        KT = persist.tile([128, NDT, M], f16, tag="KT")
        V = persist.tile([128, 2 * NDT, H * 65], f16, tag="V")
        AO = persist.tile([128, NDT, M], f16, tag="AO")    # attn out, transposed

        # ones columns of the V strips
        for mt in range(2 * NDT):
            v2 = V[:, mt, :].rearrange("p (a c) -> p a c", c=65)
            nc.gpsimd.memset(v2[:, :, 64], 1.0)

        def load_x(b):
            for dt in range(NDT):
                nc.sync.dma_start(
                    out=xTs[:, dt, b * S:(b + 1) * S],
                    in_=xT_d[dt * 128:(dt + 1) * 128, b * S:(b + 1) * S])

        def load_w_half(w_d, half):
            wt = wpool.tile([128, NDT, 512], f16, tag="W")
            src = w_d[:, half * 512:(half + 1) * 512]
            nc.sync.dma_start(out=wt,
                              in_=src.rearrange("(a p) c -> p a c", p=128))
            return wt

        # ---------- filler units (proj + output proj) ----------
        def mark_load(fn):
            fn.is_load = True
            return fn

        def proj_qk_halves(b, w_d, dst):
            """Q/K projection for batch b as [(load, units)] per d_out half."""
            state = {}
            halves = []
            for half in range(2):
                def load(half=half, w_d=w_d):
                    state[half] = load_w_half(w_d, half)
                units = []
                for ot_l in range(4):
                    ot = half * 4 + ot_l
                    for mc in (2 * b, 2 * b + 1):
                        def u(half=half, ot=ot, ot_l=ot_l, mc=mc, dst=dst):
                            wt = state[half]
                            pp = psP.tile([128, 512], f32, tag="pp")
                            for it in range(NDT):
                                nc.tensor.matmul(
                                    pp,
                                    wt[:, it, ot_l * 128:(ot_l + 1) * 128],
                                    xTs[:, it, mc * 512:(mc + 1) * 512],
                                    start=(it == 0), stop=(it == NDT - 1))
                            nc.scalar.copy(
                                out=dst[:, ot, mc * 512:(mc + 1) * 512],
                                in_=pp)
                        units.append(u)
                halves.append((mark_load(load), units))
            return halves

        def proj_v_halves(b):
            state = {}
            halves = []
            for half in range(2):
                def load(half=half):
                    state[half] = load_w_half(wv_d, half)
                units = []
                for mt_l in range(8):
                    mt = b * 8 + mt_l
                    def u(half=half, mt=mt):
                        wt = state[half]
                        pp = psP.tile([128, 512], f32, tag="pp")
                        for it in range(NDT):
                            nc.tensor.matmul(
                                pp,
                                xTs[:, it, mt * 128:(mt + 1) * 128],
                                wt[:, it, :],
                                start=(it == 0), stop=(it == NDT - 1))
                        v2 = V[:, mt, :].rearrange("p (a c) -> p a c", c=65)
                        nc.vector.tensor_copy(
                            out=v2[:, 8 * half:8 * half + 8, 0:64],
                            in_=pp.rearrange("p (a c) -> p a c", c=64))
                    units.append(u)
                halves.append((mark_load(load), units))
            return halves

        wo_state = {}

        def wo_load_units():
            def load_a():
                wo_state[0] = load_w_half(wo_d, 0)

            def load_b():
                wo_state[1] = load_w_half(wo_d, 1)
            return [mark_load(load_a), mark_load(load_b)]

        def d_units(b):
            units = []
            for mt in range(8):
                m0 = b * S + mt * 128
                for oc in range(2):
                    def u(m0=m0, oc=oc):
                        wt = wo_state[oc]
                        pp = psP.tile([128, 512], f32, tag="pp")
                        for dt in range(NDT):
                            nc.tensor.matmul(
                                pp,
                                AO[:, dt, m0:m0 + 128],
                                wt[:, dt, :],
                                start=(dt == 0), stop=(dt == NDT - 1))
                        ys = ysp.tile([128, 512], f32, tag="ys")
                        nc.vector.tensor_copy(out=ys, in_=pp)
                        nc.sync.dma_start(
                            out=y_d[m0:m0 + 128, oc * 512:(oc + 1) * 512],
                            in_=ys)
                    units.append(u)
            return units

        def interleave_halves(all_halves, tail_loads=()):
            """Flatten (load, units) halves: with wpool bufs=2, load i+2
            reuses load i's buffer, so it is emitted right after block i's
            units (all prior readers emitted) and one full block before
            its own consumers."""
            seq = []
            loads = [h[0] for h in all_halves] + list(tail_loads)
            unit_blocks = [h[1] for h in all_halves]
            seq.extend(loads[0:2])
            li = 2
            for blk in unit_blocks:
                seq.extend(blk)
                if li < len(loads):
                    seq.append(loads[li])
                    li += 1
            return seq

        # ---------- attention ----------
        def pop_filler(filler):
            # emit one PE-work unit; loads are cheap, pop through them
            while filler:
                u = filler.popleft()
                u()
                if not getattr(u, "is_load", False):
                    break

        def finish_head(b, qc, h, pso, jcol, st16):
            """Evacuate one head's unnormalized output + sum row.
            st16 row layout: 4*pair + 2*qc + (h%2), so each pair owns a
            contiguous 4-row block."""
            cols = slice(b * S + qc * 512, b * S + qc * 512 + 512)
            thq = h // 2
            if h % 2 == 0:
                nc.scalar.copy(out=AO[0:64, thq, cols],
                               in_=pso[0:64, jcol:jcol + 512])
            else:
                tmp = tmpp.tile([64, 512], f16, tag="tmp")
                nc.scalar.copy(out=tmp, in_=pso[0:64, jcol:jcol + 512])
                nc.gpsimd.dma_start(out=AO[64:128, thq, cols], in_=tmp)
            sums_t = sumsp.tile([128, 1024], f16, tag="sums")
            nc.scalar.copy(out=sums_t[64:65, 0:512],
                           in_=pso[64:65, jcol:jcol + 512])
            row = 2 * qc + (h % 2)
            nc.sync.dma_start(out=st16[row:row + 1, :],
                              in_=sums_t[64:65, 0:512])

        def norm_pair(b, pair, st16):
            """Incremental per-pair reciprocal + in-place normalize, so
            norm work pipelines into the following rounds instead of
            serializing at the end of the batch. st16 is the pair-local
            [4, 512] sums tile (rows: 2*qc + head-parity)."""
            st32 = st32p.tile([4, 512], f32, tag="st32")
            nc.vector.tensor_copy(out=st32, in_=st16)
            rc32 = st32p.tile([4, 512], f32, tag="rc32")
            nc.vector.reciprocal_approx_fast(out=rc32, in_=st32)
            rc16 = stp.tile([4, 512], f16, tag="rc16")
            nc.vector.tensor_copy(out=rc16, in_=rc32)
            for qc in range(2):
                cols = slice(b * S + qc * 512, b * S + qc * 512 + 512)
                rb = rbp.tile([128, 512], f16, tag="rb")
                r2 = rc16[2 * qc:2 * qc + 2, :]
                src = bass.AP(tensor=r2.tensor, offset=r2.offset,
                              ap=[list(r2.ap[0]), [0, 64]]
                              + [list(a) for a in r2.ap[1:]])
                nc.sync.dma_start(out=rb, in_=src)
                nc.vector.tensor_mul(AO[:, pair, cols],
                                     AO[:, pair, cols], rb)

        def emit_attn(b, filler):
            """Row-tiled head pairs, pair-outer / q-chunk-inner so pair p
            only depends on its own Q/K d_out tile and V half."""
            pending_norm = []
            for pair in range(H // 2):
                # 2-pair deferral: by now the rb broadcast for pair-2 is
                # long done, so the norm TT never blocks the DVE queue
                if len(pending_norm) >= 2:
                    norm_pair(*pending_norm.pop(0))
                st16 = stp.tile([4, 512], f16, tag="st16")
                for qc in range(2):
                    q0 = b * S + qc * 512
                    nkt = 4 * (qc + 1)
                    pso = psO.tile([128, 1024], f32, tag="pso")

                    def attn_out(kt, ex):
                        off = max(0, kt * 128 - qc * 512)
                        mtv = b * 8 + kt
                        for j in range(2):
                            h = 2 * pair + j
                            nc.tensor.matmul(
                                pso[0:65, j * 512 + off:j * 512 + 512],
                                V[:, mtv, h * 65:h * 65 + 65],
                                ex[:, j * 512 + off:(j + 1) * 512],
                                start=(kt == 0), stop=(kt == nkt - 1))

                    # kt-granular software pipeline: scores(kt) issue while
                    # exp(kt-1) runs; attn-out(kt-1) lands after scores(kt)
                    prev = None
                    for kt in range(nkt):
                        strip = psS.tile([128, 1024], f32, tag="strip")
                        kg = b * S + kt * 128
                        for j in range(2):
                            po = j * 64
                            nc.tensor.matmul(
                                strip[:, j * 512:(j + 1) * 512],
                                KT[po:po + 64, pair, kg:kg + 128],
                                QT[po:po + 64, pair, q0:q0 + 512],
                                start=True, stop=True)
                        ex = expp.tile([128, 1024], f16, tag="ex")
                        nc.scalar.activation(out=ex, in_=strip, func=EXPF,
                                             scale=0.125)
                        off = kt * 128 - qc * 512
                        if off >= 0:
                            for j in range(2):
                                sl = slice(j * 512 + off, j * 512 + off + 128)
                                nc.vector.tensor_mul(ex[:, sl], ex[:, sl],
                                                     tri01)
                        if prev is not None:
                            attn_out(*prev)
                        if kt % 2 == 1:
                            pop_filler(filler)
                        prev = (kt, ex)
                    attn_out(*prev)
                    finish_head(b, qc, 2 * pair, pso, 0, st16)
                    finish_head(b, qc, 2 * pair + 1, pso, 512, st16)
                pending_norm.append((b, pair, st16))
            while pending_norm:
                norm_pair(*pending_norm.pop(0))

        # ---------- schedule ----------
        qk0 = proj_qk_halves(0, wq_d, QT)
        kk0 = proj_qk_halves(0, wk_d, KT)
        vv0 = proj_v_halves(0)
        qk1 = proj_qk_halves(1, wq_d, QT)
        kk1 = proj_qk_halves(1, wk_d, KT)
        vv1 = proj_v_halves(1)

        load_x(0)
        load_x(1)
        # prologue: A halves of batch 0 projections (heads 0-7)
        for u in interleave_halves([qk0[0], kk0[0], vv0[0]]):
            u()
        # filler chain: B(b0), A(b1), B(b1), then Wo loads
        filler = deque(interleave_halves(
            [qk0[1], kk0[1], vv0[1],
             qk1[0], kk1[0], vv1[0],
             qk1[1], kk1[1], vv1[1]],
            tail_loads=wo_load_units()))

        emit_attn(0, filler)
        filler.extend(d_units(0))
        emit_attn(1, filler)
        while filler:
            filler.popleft()()
        for u in d_units(1):
            u()

    nc.compile()
    return nc


def _tri01():
    # tri01[dk, dq] = 1 where k <= q (allowed), else 0
    return np.triu(np.ones((128, 128), np.float16))


def _get_nc():
    if "nc" not in _CACHE:
        _CACHE["nc"] = _build_nc()
    return _CACHE["nc"]


def _in_maps(x, Wq, Wk, Wv, Wo):
    """Host-side prep: shard x, transpose + fp16-cast everything."""
    x = np.asarray(x, dtype=np.float32)
    B = x.shape[0]
    assert x.shape == (B, S, D) and B == NCORES * BPC
    shards = x.reshape(NCORES, M, D)
    wqT = np.ascontiguousarray(np.asarray(Wq, np.float32).T.astype(np.float16))
    wkT = np.ascontiguousarray(np.asarray(Wk, np.float32).T.astype(np.float16))
    wvT = np.ascontiguousarray(np.asarray(Wv, np.float32).T.astype(np.float16))
    woT = np.ascontiguousarray(np.asarray(Wo, np.float32).T.astype(np.float16))
    tri = _tri01()
    return [
        {"xT": np.ascontiguousarray(shards[c].T.astype(np.float16)),
         "WqT": wqT, "WkT": wkT, "WvT": wvT, "WoT": woT, "tri01": tri}
        for c in range(NCORES)
    ]


def kernel(x, Wq, bq, Wk, bk, Wv, bv, Wo, bo):
    from concourse.bass_utils import run_bass_kernel_spmd

    nc = _get_nc()
    in_maps = _in_maps(x, Wq, Wk, Wv, Wo)
    res = run_bass_kernel_spmd(nc, in_maps, core_ids=list(range(NCORES)))
    y = np.stack([res.results[c]["y"] for c in range(NCORES)])
    y = y.reshape(NCORES * BPC, S, D)

    # exact host-side fold of bv and bo (bq/bk are zero by problem spec)
    bias = (np.asarray(bv, np.float32) @ np.asarray(Wo, np.float32).T
            + np.asarray(bo, np.float32))
    if np.any(bias):
        y = y + bias
    return y.astype(np.float32)
